# revision 1
# baseline (speedup 1.0000x reference)
"""Trainium2 Bass kernel for the moe_routing ensemble-MLP problem.

Math (reference):
    inp_i = concat(states_i, actions_i)                 # (E, B, 40)
    h1    = leaky(inp_i @ W0_j + b0_j)                  # per (i, j)
    h2    = leaky(h1 @ W1_j + b1_j)
    p_ij  = h2 @ W2_j + b2_j                            # (E, E, B, 32)
    mean_out[j] = mean_i(p_ij) + states[j]
    var_out[j]  = var_i(p_ij, ddof=1)

Strategy: data-parallel over B across 8 cores (B_loc = 2048/core).
Per core, everything is computed feature-major ([feat, tokens] tiles):
  - inputs are PE-transposed once into per-i inpT tiles [40, B_loc]
    (separate tiles so the main loop overlaps the input phase)
  - mm0/mm1 stream tokens through stationary per-member weights (f32r,
    1 cyc/row at N=512)
  - mm2 uses a zero-padded stationary per token-quarter: W2[j] embedded
    at col-block 32t of a zeroed [128,128], so 4 accumulating matmuls
    stack the 4 quarters across all 128 PSUM partitions; a second
    accumulating pass computes sum_i(preds) in PSUM for free-ish
  - leaky relu is balanced between ACT (Prelu, 1 op) and DVE (2 ops)
    at 512-column granularity on the zero-bias fast path
  - final [32, token] -> [token, 32] flip via DVE 32x32 stream transpose
    + a strided DMA store
"""

import math

import numpy as np

import concourse.bass as bass
import concourse.bacc as bacc
import concourse.mybir as mybir
import concourse.tile as tile
from concourse.bass_utils import run_bass_kernel_spmd

F32 = mybir.dt.float32
F32R = mybir.dt.float32r

E, DS, DA, H = 8, 32, 8, 128
DIN = DS + DA          # 40
B = 16384
N_CORES = 8
B_LOC = B // N_CORES   # 2048
QT = 512               # token quarter = drain granularity
NQ = B_LOC // QT       # 4
CH = 1024              # layer-2 pipeline chunk
NCH = B_LOC // CH      # 2
SLOPE = 0.01

# drain (leaky) units routed to DVE when (ctr % MOD) in SET
DVE_LK_MOD = 7
DVE_LK_SET = (1, 4)
PE_SUM = True          # accumulate sum_i(preds) via a 2nd mm2 PSUM pass


def build_kernel(zero_bias: bool, repeat: int = 1):
    nc = bacc.Bacc("TRN2")

    states_d = nc.dram_tensor("states", [E, B_LOC, DS], F32, kind="ExternalInput")
    actions_d = nc.dram_tensor("actions", [E, B_LOC, DA], F32, kind="ExternalInput")
    w0_d = nc.dram_tensor("w0", [E, DIN, H], F32, kind="ExternalInput")
    b0_d = nc.dram_tensor("b0", [E, H], F32, kind="ExternalInput")
    w1_d = nc.dram_tensor("w1", [E, H, H], F32, kind="ExternalInput")
    b1_d = nc.dram_tensor("b1", [E, H], F32, kind="ExternalInput")
    w2_d = nc.dram_tensor("w2", [E, H, DS], F32, kind="ExternalInput")
    b2_d = nc.dram_tensor("b2", [E, DS], F32, kind="ExternalInput")
    mean_d = nc.dram_tensor("mean_out", [E, B_LOC, DS], F32, kind="ExternalOutput")
    var_d = nc.dram_tensor("var_out", [E, B_LOC, DS], F32, kind="ExternalOutput")

    ident_d = nc.inline_tensor(np.eye(128, dtype=np.float32), name="ident")

    pe_sum = PE_SUM and zero_bias
    lk_ctr = [0]

    def leaky512(out_ap, psum_ap, pool, bias_ap):
        n = lk_ctr[0]
        lk_ctr[0] += 1
        if bias_ap is None and (n % DVE_LK_MOD) in DVE_LK_SET:
            t = pool.tile([128, QT], F32, tag="lk")
            nc.vector.tensor_scalar_mul(t[:, :], psum_ap, SLOPE)
            nc.vector.tensor_tensor(out_ap, t[:, :], psum_ap, mybir.AluOpType.max)
        else:
            nc.scalar.activation(
                out_ap, psum_ap, mybir.ActivationFunctionType.Prelu,
                bias=0.0 if bias_ap is None else bias_ap, alpha=SLOPE,
            )

    with tile.TileContext(nc) as tc:
        with (
            tc.tile_pool(name="wpool", bufs=1) as wpool,
            tc.tile_pool(name="big", bufs=1) as big,
            tc.tile_pool(name="io", bufs=1) as io,
            tc.tile_pool(name="hs", bufs=5) as hs,
            tc.tile_pool(name="fin", bufs=2) as fin,
        ):
            # ---- static weights / constants in SBUF ----
            w0s = wpool.tile([DIN, E * H], F32R)
            w1s = wpool.tile([H, E * H], F32R)
            # W2[j] embedded at col-block 32t of a zeroed [128,128] per
            # (j, t); block for (j, t) starts at col j*512 + 160*t.
            w2z = wpool.tile([H, E * 4 * H], F32R)
            ident = wpool.tile([128, 128], F32)

            nc.gpsimd.memset(w2z[:, :].bitcast(F32), 0.0)
            nc.sync.dma_start(ident[:, :], ident_d[:, :])
            nat_tiles = []
            for i in range(E):
                nat = io.tile([128, 16 * DIN], F32, tag=f"nat{i}",
                              name=f"nat{i}")
                natv = nat[:, :].rearrange("p (m d) -> p m d", m=16)
                nc.sync.dma_start(
                    natv[:, :, 0:DS],
                    states_d[i].rearrange("(m p) d -> p m d", p=128),
                )
                nc.sync.dma_start(
                    natv[:, :, DS:DIN],
                    actions_d[i].rearrange("(m p) d -> p m d", p=128),
                )
                nat_tiles.append(nat)
            nc.sync.dma_start(
                w0s[:, :].rearrange("d (j h) -> d j h", j=E),
                w0_d[:, :, :].rearrange("j d h -> d j h").bitcast(F32R),
            )
            nc.sync.dma_start(
                w1s[:, :].rearrange("d (j h) -> d j h", j=E),
                w1_d[:, :, :].rearrange("j d h -> d j h").bitcast(F32R),
            )
            w2zv = w2z[:, :].rearrange("d (j q) -> d j q", j=E)
            for t in range(4):
                nc.sync.dma_start(
                    w2zv[:, :, 160 * t: 160 * t + DS],
                    w2_d[:, :, :].rearrange("j d k -> d j k").bitcast(F32R),
                )
            if not zero_bias:
                b0s = wpool.tile([H, E], F32)
                b1s = wpool.tile([H, E], F32)
                b2r = wpool.tile([H, E], F32)  # b2[j] on 32t+k partitions
                nc.sync.dma_start(b0s[:, :], b0_d[:, :].rearrange("j h -> h j"))
                nc.sync.dma_start(b1s[:, :], b1_d[:, :].rearrange("j h -> h j"))
                for t in range(4):
                    nc.sync.dma_start(
                        b2r[32 * t: 32 * (t + 1), :],
                        b2_d[:, :].rearrange("j k -> k j"),
                    )

            # ---- accumulators ----
            sum_acc = None if pe_sum else big.tile([128, E * QT], F32)
            sumsq_acc = big.tile([128, E * QT], F32)
            # per-i transposed input, feature-major
            inpT = [
                big.tile([DIN, B_LOC], F32R, tag=f"inpT{i}", name=f"inpT{i}")
                for i in range(E)
            ]

            # ---- phase 1 is emitted interleaved with the start of
            # phase 2 so the per-engine static schedules overlap ----
            def make_input_emitter(tp_psum):
                def emit_input_phase(i):
                    nat = nat_tiles[i]
                    for g in range(NQ):  # groups of 512 tokens
                        pt = tp_psum.tile([DIN, QT], F32, tag="pt", name=f"pt{i}_{g}")
                        for m in range(4):
                            mm = g * 4 + m
                            nc.tensor.transpose(
                                pt[:, m * 128:(m + 1) * 128],
                                nat[:, mm * DIN:(mm + 1) * DIN],
                                ident[:, :],
                            )
                        if g % 2 == 0:
                            nc.vector.tensor_copy(
                                inpT[i][:, g * QT:(g + 1) * QT], pt[:, :]
                            )
                        else:
                            nc.scalar.copy(
                                inpT[i][:, g * QT:(g + 1) * QT], pt[:, :]
                            )
                return emit_input_phase

            with tc.tile_pool(name="tp_psum", bufs=3, space="PSUM") as tp_psum:
                emit = make_input_emitter(tp_psum)
                for i in range(E):
                    emit(i)

            with (
                tc.tile_pool(name="ps_h1", bufs=2, space="PSUM") as ps_h1,
                tc.tile_pool(name="ps_h2", bufs=2, space="PSUM") as ps_h2,
                tc.tile_pool(name="ps_p", bufs=1, space="PSUM") as ps_p,
                tc.tile_pool(name="ps_s", bufs=1, space="PSUM") as ps_s,
            ):
              pending_fin = []
              for _rep in range(repeat):
                for j in range(E):
                    jH = j * H
                    psum_sum = None
                    if pe_sum:
                        psum_sum = ps_s.tile([128, QT], F32, tag="psum",
                                             name="psum_sum")
                    for i in range(E):
                        pp = ps_p.tile([128, QT], F32, tag="pp")
                        for c in range(NCH):
                            h2p = ps_h2.tile([128, CH], F32, tag="h2p")
                            for s in range(CH // QT):
                                base = c * CH + s * QT
                                h1p = ps_h1.tile([128, QT], F32, tag="h1p")
                                nc.tensor.matmul(
                                    h1p[:, :],
                                    w0s[:, jH:jH + H],
                                    inpT[i][:, base:base + QT],
                                )
                                h1s = hs.tile([128, QT], F32R, tag="h1s")
                                leaky512(
                                    h1s[:, :], h1p[:, :], hs,
                                    None if zero_bias else b0s[:, j:j + 1],
                                )
                                nc.tensor.matmul(
                                    h2p[:, s * QT:(s + 1) * QT],
                                    w1s[:, jH:jH + H],
                                    h1s[:, :],
                                )
                            h2s = hs.tile([128, CH], F32R, tag="h2s")
                            for s in range(CH // QT):
                                sl = (slice(None), slice(s * QT, (s + 1) * QT))
                                leaky512(
                                    h2s[sl], h2p[sl], hs,
                                    None if zero_bias else b1s[:, j:j + 1],
                                )
                            for t2 in range(CH // QT):
                                T = c * (CH // QT) + t2
                                zc = (j * 4 + T) * H
                                rhs = h2s[:, t2 * QT:(t2 + 1) * QT]
                                nc.tensor.matmul(
                                    pp[:, :], w2z[:, zc:zc + H], rhs,
                                    start=(T == 0), stop=(T == 3),
                                )
                                if pe_sum:
                                    nc.tensor.matmul(
                                        psum_sum[:, :], w2z[:, zc:zc + H], rhs,
                                        start=(i == 0 and T == 0),
                                        stop=(i == E - 1 and T == 3),
                                        skip_group_check=True,
                                    )
                        # accumulate sumsq (and sum when not on PE) over i
                        acc_sl = (slice(None), slice(j * QT, (j + 1) * QT))
                        sq_bias = 0.0 if zero_bias else b2r[:, j:j + 1]
                        if not pe_sum:
                            if i == 0:
                                nc.vector.tensor_copy(sum_acc[acc_sl], pp[:, :])
                            else:
                                nc.vector.tensor_tensor(
                                    sum_acc[acc_sl], sum_acc[acc_sl], pp[:, :],
                                    mybir.AluOpType.add,
                                )
                        if i == 1 and pending_fin:
                            pending_fin.pop(0)()
                        if i == 4:
                            rep = fin.tile([128, QT], F32, tag="rep")
                            for t in range(4):
                                nc.sync.dma_start(
                                    rep[32 * t:32 * (t + 1), :],
                                    inpT[j][:DS, t * QT:(t + 1) * QT
                                            ].bitcast(F32),
                                )
                        if i == 0:
                            nc.scalar.activation(
                                sumsq_acc[acc_sl], pp[:, :],
                                mybir.ActivationFunctionType.Square,
                                bias=sq_bias,
                            )
                        else:
                            sq = hs.tile([128, QT], F32, tag="sq")
                            nc.scalar.activation(
                                sq[:, :], pp[:, :],
                                mybir.ActivationFunctionType.Square,
                                bias=sq_bias,
                            )
                            nc.gpsimd.tensor_tensor(
                                sumsq_acc[acc_sl], sumsq_acc[acc_sl], sq[:, :],
                                mybir.AluOpType.add,
                            )

                    # ---- finalize member j: m_t now (frees psum_sum);
                    # the rest is deferred into (j+1, i=1) so it does not
                    # head-of-line block ACT/DVE at the j boundary ----
                    acc_sl = (slice(None), slice(j * QT, (j + 1) * QT))
                    m_t = fin.tile([128, QT], F32, tag="m_t")
                    msrc = psum_sum[:, :] if pe_sum else sum_acc[acc_sl]
                    if zero_bias:
                        nc.vector.tensor_scalar(
                            m_t[:, :], msrc, 1.0 / E, None, mybir.AluOpType.mult,
                        )
                    else:
                        nc.vector.tensor_scalar(
                            m_t[:, :], msrc, 1.0 / E, b2r[:, j:j + 1],
                            mybir.AluOpType.mult, mybir.AluOpType.add,
                        )

                    def fin_tail(j=j, m_t=m_t, rep=rep, acc_sl=acc_sl):
                        mean_st = fin.tile([128, QT], F32, tag="mean_st",
                                           name=f"mean_st{j}")
                        nc.gpsimd.tensor_tensor(
                            mean_st[:, :], m_t[:, :], rep[:, :],
                            mybir.AluOpType.add
                        )
                        msq = fin.tile([128, QT], F32, tag="msq",
                                       name=f"msq{j}")
                        nc.scalar.activation(
                            msq[:, :], m_t[:, :],
                            mybir.ActivationFunctionType.Square,
                            scale=math.sqrt(E / (E - 1.0)),
                        )
                        var_st = fin.tile([128, QT], F32, tag="var_st",
                                          name=f"var_st{j}")
                        nc.vector.scalar_tensor_tensor(
                            var_st[:, :], sumsq_acc[acc_sl], 1.0 / (E - 1.0),
                            msq[:, :],
                            mybir.AluOpType.mult, mybir.AluOpType.subtract,
                        )
                        xm = fin.tile([128, QT], F32, tag="xm", name=f"xm{j}")
                        nc.vector.transpose(xm[:, :], mean_st[:, :])
                        xv = fin.tile([128, QT], F32, tag="xv", name=f"xv{j}")
                        nc.vector.transpose(xv[:, :], var_st[:, :])
                        for t in range(4):
                            tok = slice(t * QT, (t + 1) * QT)
                            prt = slice(32 * t, 32 * (t + 1))
                            nc.sync.dma_start(
                                mean_d[j][tok].rearrange("(u p) k -> p u k", p=32),
                                xm[prt, :].rearrange("p (u k) -> p u k", k=DS),
                            )
                            nc.sync.dma_start(
                                var_d[j][tok].rearrange("(u p) k -> p u k", p=32),
                                xv[prt, :].rearrange("p (u k) -> p u k", k=DS),
                            )

                    pending_fin.append(fin_tail)

            for f in pending_fin:
                f()
            pending_fin.clear()

    nc.compile()
    return nc


_NC_CACHE = {}


def kernel(states, actions, W0, b0, W1, b1, W2, b2):
    states = np.ascontiguousarray(states, dtype=np.float32)
    actions = np.ascontiguousarray(actions, dtype=np.float32)
    W0 = np.ascontiguousarray(W0, dtype=np.float32)
    W1 = np.ascontiguousarray(W1, dtype=np.float32)
    W2 = np.ascontiguousarray(W2, dtype=np.float32)
    b0 = np.ascontiguousarray(b0, dtype=np.float32)
    b1 = np.ascontiguousarray(b1, dtype=np.float32)
    b2 = np.ascontiguousarray(b2, dtype=np.float32)

    zb = not (b0.any() or b1.any() or b2.any())
    if zb not in _NC_CACHE:
        _NC_CACHE[zb] = build_kernel(zb)
    nc = _NC_CACHE[zb]

    in_maps = []
    for c in range(N_CORES):
        sl = slice(c * B_LOC, (c + 1) * B_LOC)
        in_maps.append({
            "states": np.ascontiguousarray(states[:, sl, :]),
            "actions": np.ascontiguousarray(actions[:, sl, :]),
            "w0": W0, "b0": b0, "w1": W1, "b1": b1, "w2": W2, "b2": b2,
        })

    res = run_bass_kernel_spmd(nc, in_maps, list(range(N_CORES)))
    mean = np.concatenate([r["mean_out"] for r in res.results], axis=1)
    var = np.concatenate([r["var_out"] for r in res.results], axis=1)
    return mean, var

